# revision 6
# baseline (speedup 1.0000x reference)
"""Sequence-parallel attention-context kernel for 8 TRN2 NeuronCores.

reference math:
    v      = W @ decoder_hidden.T                    # [E]
    scores = encoder_hiddens @ v                     # [S]
    wts    = softmax(scores)                         # [S]
    out    = wts @ encoder_hiddens                   # [1, E]

Distribution (SPMD over 8 cores):
  - encoder_hiddens sharded along seq (2048 rows/core).
  - W sharded along its rows (256 rows/core) -> each core computes a slice
    of v, AllGather -> full v everywhere.
  - softmax normalizer handled with a global "safe max" M = 5*||v||
    (scores ~ N(0, ||v||^2) so realized max ~ 4.4*||v||; exp(s - M) stays
    in fp32 range; identical M on every core so the cross-core combine is
    a plain sum).  Each core computes local Z_partials and local context
    partials, AllGather [17,128] fp32 payload, every core reduces and
    writes the identical [1, 2048] output.

Engine plan per core:
  - DMA (SWDGE): f32->bf16 cast while loading enc tiles / W / broadcasts.
  - VectorE: fused multiply+reduce (tensor_tensor_reduce) for v-slice and
    for per-tile scores (contraction over the free axis).
  - ScalarE: exp(score - M), copies, square/sqrt for ||v||.
  - TensorE: context matmuls (contraction over seq partitions), the
    [128,17]->[17,128] transpose, and the 8-way combine matmul.
"""

import numpy as np

NCORES = 8
S, E, D = 16384, 2048, 2048
SSH = S // NCORES          # 2048 seq rows per core
ESH = E // NCORES          # 256 rows of W per core
NT = SSH // 128            # 16 s-tiles per core
NEC = E // 128             # 16 e-chunks
SAFE_MAX_MULT = 5.0


def _build_nc():
    from concourse import bass, mybir, tile, bacc

    f32 = mybir.dt.float32
    bf16 = mybir.dt.bfloat16
    AOT = mybir.AluOpType
    AFT = mybir.ActivationFunctionType

    nc = bacc.Bacc(None, target_bir_lowering=False, debug=False,
                   num_devices=NCORES)

    enc_ext = nc.declare_dram_parameter("encoder_hiddens", [SSH, E], f32,
                                        isOutput=False)
    dec_ext = nc.declare_dram_parameter("decoder_hidden", [1, D], f32,
                                        isOutput=False)
    w_ext = nc.declare_dram_parameter("W", [128, 2 * D], f32, isOutput=False)
    out_ext = nc.declare_dram_parameter("out", [1, E], f32, isOutput=True)

    ident_dram = nc.inline_tensor(np.eye(128, dtype=np.float32), name="ident")
    ones8_dram = nc.inline_tensor(np.ones((8, 1), dtype=np.float32),
                                  name="ones8")

    rg = [list(range(NCORES))]

    with tile.TileContext(nc) as tc:
        with (
            tc.tile_pool(name="encp", bufs=NT) as encp,
            tc.tile_pool(name="scr", bufs=2) as scr,
            tc.tile_pool(name="cst", bufs=1) as cst,
            tc.tile_pool(name="sm", bufs=1) as sm,
            tc.tile_pool(name="psum", bufs=1, space="PSUM") as psp,
            tc.tile_pool(name="dram", bufs=1, space="DRAM") as dram,
        ):
            # ---- DRAM bounce buffers for the collectives
            v_in_dram = dram.tile([128, 2], f32)
            v_all_dram = dram.tile([1, E], f32)
            zc_in_dram = dram.tile([NT + 1, 128], f32)
            zc_all_dram = dram.tile([NCORES, (NT + 1) * 128], f32)

            # ---- constants
            ident_sb = cst.tile([128, 128], f32)
            nc.sync.dma_start(out=ident_sb[:], in_=ident_dram[:, :])
            ones8_sb = cst.tile([8, 1], f32)
            nc.sync.dma_start(out=ones8_sb[:], in_=ones8_dram[:, :])

            # ---- v-slice computation: v[e] = sum_d W[e, d] * dec[d]
            # W slice rows are (2p + j) within this core's 256-row block.
            dec_bc = cst.tile([128, D], bf16)
            nc.gpsimd.dma_start(out=dec_bc[:],
                                in_=dec_ext.ap().partition_broadcast(128))
            wmat_sb = cst.tile([128, 2 * D], bf16)
            nc.gpsimd.dma_start(out=wmat_sb[:], in_=w_ext[:, :])

            vparts = sm.tile([128, 2], f32)
            for j in range(2):
                ttr_scratch = scr.tile([128, D], bf16, tag="ttrs")
                nc.vector.scalar_tensor_tensor(
                    out=ttr_scratch[:],
                    in0=wmat_sb[:, j * D:(j + 1) * D],
                    scalar=1.0,
                    in1=dec_bc[:],
                    op0=AOT.mult,
                    op1=AOT.mult,
                    accum_out=vparts[:, j:j + 1],
                )
            nc.scalar.dma_start(out=v_in_dram[:], in_=vparts[:])

            nc.gpsimd.collective_compute(
                "AllGather",
                AOT.bypass,
                replica_groups=rg,
                ins=[v_in_dram.opt()],
                outs=[v_all_dram.opt()],
            )

            v_bc = cst.tile([128, E], bf16)
            nc.gpsimd.dma_start(out=v_bc[:],
                                in_=v_all_dram[:].partition_broadcast(128))

            # ---- safe max:  m_neg = -SAFE_MAX_MULT * ||v||   (per partition)
            sq_scratch = scr.tile([128, E], bf16, tag="sqs")
            vsq = sm.tile([128, 1], f32)
            nc.scalar.activation(out=sq_scratch[:], in_=v_bc[:],
                                 func=AFT.Square, accum_out=vsq[:])
            vstd = sm.tile([128, 1], f32)
            nc.scalar.activation(out=vstd[:], in_=vsq[:], func=AFT.Sqrt)
            m_neg = sm.tile([128, 1], f32)
            nc.vector.tensor_scalar_mul(out=m_neg[:], in0=vstd[:],
                                        scalar1=-SAFE_MAX_MULT)

            # ---- main pipeline over the 16 seq tiles
            enc_tiles = []
            for t in range(NT):
                et = encp.tile([128, E], bf16, tag="enc")
                enc_tiles.append(et)
                nc.gpsimd.dma_start(out=et[:],
                                    in_=enc_ext[t * 128:(t + 1) * 128, :])

            scores_sb = sm.tile([128, NT], f32)
            wexp_sb = sm.tile([128, NT], bf16)
            ctx_ps = psp.tile([128, NEC], f32)

            for t in range(NT):
                ttr_scratch = scr.tile([128, E], bf16, tag="ttrs")
                nc.vector.scalar_tensor_tensor(
                    out=ttr_scratch[:],
                    in0=enc_tiles[t][:],
                    scalar=1.0,
                    in1=v_bc[:],
                    op0=AOT.mult,
                    op1=AOT.mult,
                    accum_out=scores_sb[:, t:t + 1],
                )
                nc.scalar.activation(out=wexp_sb[:, t:t + 1],
                                     in_=scores_sb[:, t:t + 1],
                                     func=AFT.Exp,
                                     bias=m_neg[:],
                                     scale=1.0)
                for c in range(NEC):
                    nc.tensor.matmul(
                        out=ctx_ps[:, c:c + 1],
                        lhsT=enc_tiles[t][:, c * 128:(c + 1) * 128],
                        rhs=wexp_sb[:, t:t + 1],
                        start=(t == 0 and c == 0),
                        stop=(t == NT - 1 and c == NEC - 1),
                    )

            # ---- local tail: pack [context(16 cols) | z_partials(1 col)]
            cw_sb = sm.tile([128, NT + 1], f32)
            nc.scalar.activation(out=cw_sb[:, 0:NT], in_=ctx_ps[:],
                                 func=AFT.Copy, bias=0.0, scale=1.0)
            nc.vector.reduce_sum(out=cw_sb[:, NT:NT + 1], in_=wexp_sb[:],
                                 axis=mybir.AxisListType.X)

            zc_ps = psp.tile([NT + 1, 128], f32)
            nc.tensor.transpose(out=zc_ps[:], in_=cw_sb[:], identity=ident_sb[:])
            zc_sb = sm.tile([NT + 1, 128], f32)
            nc.scalar.activation(out=zc_sb[:], in_=zc_ps[:],
                                 func=AFT.Copy, bias=0.0, scale=1.0)
            nc.scalar.dma_start(out=zc_in_dram[:], in_=zc_sb[:])

            nc.gpsimd.collective_compute(
                "AllGather",
                AOT.bypass,
                replica_groups=rg,
                ins=[zc_in_dram.opt()],
                outs=[zc_all_dram.opt()],
            )

            # ---- combine: sum the 8 payloads, divide by Z, write out
            ag_sb = sm.tile([NCORES, (NT + 1) * 128], f32)
            nc.sync.dma_start(out=ag_sb[:], in_=zc_all_dram[:])

            sum_ps = psp.tile([1, (NT + 1) * 128], f32)
            chunks = [(0, 512), (512, 512), (1024, 512), (1536, 512),
                      (2048, 128)]
            for a, n in chunks:
                nc.tensor.matmul(
                    out=sum_ps[0:1, a:a + n],
                    lhsT=ones8_sb[:],
                    rhs=ag_sb[:, a:a + n],
                    start=True,
                    stop=True,
                )

            z_tot = sm.tile([1, 1], f32)
            nc.vector.reduce_sum(out=z_tot[:], in_=sum_ps[0:1, E:E + 128],
                                 axis=mybir.AxisListType.X)
            rz = sm.tile([1, 1], f32)
            nc.vector.reciprocal(out=rz[:], in_=z_tot[:])

            res_sb = sm.tile([1, E], f32)
            nc.scalar.activation(out=res_sb[:], in_=sum_ps[0:1, 0:E],
                                 func=AFT.Copy, bias=0.0, scale=rz[:])
            nc.sync.dma_start(out=out_ext[:, :], in_=res_sb[:])

    nc.compile()
    return nc


_CACHED_NC = None


def _get_nc():
    global _CACHED_NC
    if _CACHED_NC is None:
        _CACHED_NC = _build_nc()
    return _CACHED_NC


def _make_in_maps(encoder_hiddens, decoder_hidden, W):
    in_maps = []
    for i in range(NCORES):
        wsl = np.ascontiguousarray(
            W[i * ESH:(i + 1) * ESH, :]).reshape(128, 2 * D)
        in_maps.append({
            "encoder_hiddens": np.ascontiguousarray(
                encoder_hiddens[i * SSH:(i + 1) * SSH, :]),
            "decoder_hidden": np.ascontiguousarray(decoder_hidden),
            "W": wsl,
        })
    return in_maps


def kernel(encoder_hiddens, decoder_hidden, W):
    from concourse.bass_utils import run_bass_kernel_spmd

    encoder_hiddens = np.asarray(encoder_hiddens, dtype=np.float32)
    decoder_hidden = np.asarray(decoder_hidden, dtype=np.float32)
    W = np.asarray(W, dtype=np.float32)

    nc = _get_nc()
    in_maps = _make_in_maps(encoder_hiddens, decoder_hidden, W)
    res = run_bass_kernel_spmd(nc, in_maps, core_ids=list(range(NCORES)))
    return np.asarray(res.results[0]["out"], dtype=np.float32)
